# revision 30
# baseline (speedup 1.0000x reference)
"""Trainium2 kernel for nn_Dense_RBS_density_3D.

The reference applies 39 RBS gates sequentially to a batch of 64 density
matrices: rho <- U rho U^T. The gates compose into one orthogonal matrix
V = U_38 @ ... @ U_0, so the output is V rho V^T per batch element.

Host side: build V in fp64, then apply a reverse-Cuthill-McKee permutation of
the 780-dim basis (computed from V's significance mask) to both rows and
columns. RCM tightens V's band so each 128-row contraction tile of V^T
touches a narrower, contiguous set of output columns. Per (k-tile, PSUM bank)
the host keeps the single column interval carrying all significant mass
(mass-based threshold, DROP_BUDGET relative Frobenius perturbation).

Device side (8 NeuronCores, data-parallel over batch): per batch element
compute Y = V X V^T as two transpose-free passes of f(Z) = Z^T @ V^T
(lhsT = Z with contraction on partitions, rhs = V^T):

    Y = f(f(X))   since (X^T V^T)^T V^T = V X V^T

bf16 operands with fp32 PSUM accumulation; per-element has_written bits make
ragged-interval accumulation exact. The ragged 12-row tail chunks (780 = 6*128
+ 12) are packed: pass-1 tails of 4 elements run concurrently as 32-aligned
column-group tiles of the PE array (tile_position), and pass-2 tails of all 8
elements run as one 96-column sweep reading a strided AP over the per-k-chunk
PT tiles (which hold all 8 elements side by side).
"""

import numpy as np
import ml_dtypes

D = 780           # binom(40, 2)
N_GATES = 39
B_TOTAL = 64
N_CORES = 8
B_LOC = B_TOTAL // N_CORES
P = 128
KT = (D + P - 1) // P          # 7 k-chunks: 6x128 + 12
LAST = D - (KT - 1) * P        # 12
FULL = (KT - 1) * P            # 768
CHUNKS = [(i * P, min(P, D - i * P)) for i in range(KT)]
BANKS = [(0, 512), (512, D)]   # PSUM fp32 bank col ranges
DROP_BUDGET = 5e-3             # allowed relative Frobenius perturbation of V

_CACHE = {}


def _build_V(angles, Bmat):
    """V = U_38 @ ... @ U_0 in float64 (B[g,j,i]==+1 marks the pair)."""
    V = np.eye(D, dtype=np.float64)
    for g in range(N_GATES):
        jj, ii = np.nonzero(Bmat[g] > 0.5)
        c = np.cos(float(angles[g]))
        s = np.sin(float(angles[g]))
        Vi = V[ii, :].copy()
        Vj = V[jj, :].copy()
        V[ii, :] = c * Vi - s * Vj
        V[jj, :] = s * Vi + c * Vj
    return V


def _rcm_perm(V):
    """Bandwidth-minimizing permutation of the significance graph."""
    sq = np.sort((V ** 2).ravel())
    cs = np.cumsum(sq)
    pos = np.searchsorted(cs, (DROP_BUDGET ** 2) * cs[-1])
    thr = sq[pos - 1] if pos > 0 else -1.0
    mask = (V ** 2) > thr
    try:
        from scipy.sparse import csr_matrix
        from scipy.sparse.csgraph import reverse_cuthill_mckee
        perm = np.asarray(reverse_cuthill_mckee(csr_matrix(mask | mask.T)),
                          np.int64)
        if len(perm) == D:
            return perm
    except Exception:
        pass
    return np.arange(D)


def _plan_intervals(Vp):
    """Per (k-tile, PSUM bank): [c0, c1) column interval of Vp^T holding all
    significant mass, or None."""
    VT = Vp.T
    sliver = np.zeros((KT, D))
    for kc, (k0, ksz) in enumerate(CHUNKS):
        sliver[kc] = (VT[k0:k0 + ksz, :] ** 2).sum(axis=0)
    tot = sliver.sum()
    flat = np.sort(sliver.ravel())
    csum = np.cumsum(flat)
    pos = np.searchsorted(csum, DROP_BUDGET ** 2 * tot)
    thr = flat[pos - 1] if pos > 0 else -1.0
    sig = sliver > thr

    intervals = []
    for kc in range(KT):
        row = []
        for b0, b1 in BANKS:
            cols = np.nonzero(sig[kc, b0:b1])[0]
            if len(cols) == 0:
                row.append(None)
                continue
            c0 = int(b0 + cols[0]) & ~1
            c1 = min(b1, (int(b0 + cols[-1]) + 2) & ~1)
            row.append((c0, c1))
        intervals.append(row)

    # every column must be covered by at least one kept interval
    covered = np.zeros(D, bool)
    for row in intervals:
        for iv in row:
            if iv is not None:
                covered[iv[0]:iv[1]] = True
    if not covered.all():
        for bi, (b0, b1) in enumerate(BANKS):
            if not covered[b0:b1].all():
                kc = int(sliver[:, b0:b1].sum(axis=1).argmax())
                intervals[kc][bi] = (b0, b1)
    return intervals


def _build_program(intervals):
    import concourse.bacc as bacc
    import concourse.mybir as mybir
    import concourse.tile as tile

    nc = bacc.Bacc("TRN2", target_bir_lowering=False, debug=False,
                   num_devices=N_CORES)
    x = nc.dram_tensor("x", [B_LOC, D, D], mybir.dt.bfloat16,
                       kind="ExternalInput").ap()
    vt = nc.dram_tensor("vt", [D, D], mybir.dt.bfloat16,
                        kind="ExternalInput").ap()
    y = nc.dram_tensor("y", [B_LOC, D, D], mybir.dt.bfloat16,
                       kind="ExternalOutput").ap()

    bf16 = mybir.dt.bfloat16
    f32 = mybir.dt.float32

    kept = [(kc, bi, iv[0], iv[1])
            for kc in range(KT) for bi, iv in enumerate(intervals[kc])
            if iv is not None]
    first_kc = {}
    last_kc = {}
    for kc, bi, _, _ in kept:
        first_kc.setdefault(bi, kc)
        last_kc[bi] = kc

    with tile.TileContext(nc) as tc:
        with (
            tc.tile_pool(name="vtp", bufs=1) as vtp,
            tc.tile_pool(name="xb", bufs=6) as xbp,
            tc.tile_pool(name="pt", bufs=1) as ptp,
            tc.tile_pool(name="yo", bufs=4) as yop,
            tc.tile_pool(name="wup", bufs=1) as wup,
            tc.tile_pool(name="ps1", bufs=2, space="PSUM") as ps1p,
            tc.tile_pool(name="ps2", bufs=2, space="PSUM") as ps2p,
        ):
            # PE warmup while the first DMAs land
            wz = wup.tile([P, 512], bf16)
            nc.vector.memset(wz[:], 0.0)
            ps_w = ps1p.tile([P, D], f32, tag="ps1", name="ps_w")
            for _ in range(22):
                nc.tensor.matmul(ps_w[:, :512], wz[:, :P], wz[:, :512],
                                 start=True, stop=True)

            # V^T resident in SBUF, k-partitioned: vt_sb[p, kc, n]
            vt_sb = vtp.tile([P, KT, D], bf16)
            nc.any.memzero(vt_sb[:, KT - 1, :])
            nc.sync.dma_start(
                vt_sb[:, : KT - 1, :],
                vt[:FULL, :].rearrange("(kc p) n -> p kc n", p=P),
            )
            nc.sync.dma_start(vt_sb[:LAST, KT - 1, :], vt[FULL:, :])

            # PT mega-tiles: one per k-chunk, all 8 elements side by side.
            # ptk[kc][p, e, n] = PT_e[kc*128 + p, n]
            ptk = [ptp.tile([P, B_LOC, D], bf16, tag=f"pt{mc}",
                            name=f"ptk{mc}")
                   for mc in range(KT)]
            nc.any.memzero(ptk[KT - 1][:])   # pad partitions of ragged chunk
            # contiguous copies of PT's last-12 columns (pass-2 tail weights
            # need a single-free-dim AP): ptail[kc][p, e*12+t]
            ptail = [ptp.tile([P, B_LOC * LAST], bf16, tag=f"ptt{mc}",
                              name=f"ptail{mc}")
                     for mc in range(KT)]
            nc.any.memzero(ptail[KT - 1][:])

            copy_idx = 0

            def evac(out_ap, psum_ap, small=False):
                # alternate PSUM evacuation between ScalarE and VectorE
                nonlocal copy_idx
                if copy_idx % 2 == 0:
                    nc.scalar.copy(out_ap, psum_ap)
                else:
                    nc.vector.tensor_copy(out=out_ap, in_=psum_ap)
                copy_idx += 1

            def pass_mms(ps, src_fn, msz, tile_position=None):
                for kc, bi, c0, c1 in kept:
                    kw = {}
                    if tile_position is not None:
                        kw["tile_position"] = tile_position
                    nc.tensor.matmul(
                        ps[:, c0:c1] if msz is None else ps[:msz, c0:c1],
                        src_fn(kc),
                        vt_sb[:, kc, c0:c1],
                        start=(kc == first_kc[bi]),
                        stop=(kc == last_kc[bi]),
                        **kw,
                    )

            def load_x(b):
                xb_bf = xbp.tile([P, KT, D], bf16, tag="xb")
                nc.any.memzero(xb_bf[:, KT - 1, :])
                nc.sync.dma_start(
                    xb_bf[:, : KT - 1, :],
                    x[b, :FULL, :].rearrange("(kc p) n -> p kc n", p=P),
                )
                nc.sync.dma_start(xb_bf[:LAST, KT - 1, :], x[b, FULL:, :])
                return xb_bf

            xtiles = {}
            for b in range(min(3, B_LOC)):
                xtiles[b] = load_x(b)

            for b in range(B_LOC):
                xb_bf = xtiles[b]

                # ---- pass 1 (main): PT_b[m, :] for 6 full m-chunks ----
                for mc in range(KT - 1):
                    m0 = mc * P
                    ps = ps1p.tile([P, D], f32, tag="ps1", name="ps_t")
                    pass_mms(ps,
                             lambda kc: xb_bf[:, kc, m0:m0 + P], P)
                    evac(ptk[mc][:, b, :], ps[:, :])
                    evac(ptail[mc][:, b * LAST:(b + 1) * LAST],
                         ps[:, FULL:], small=True)

                # ---- pass 1 (tails): pack 4 elements as PE column-group
                # tiles; tails of elems 4q..4q+3 run after elem 4q+3's mains.
                if b % 4 == 3:
                    q0 = b - 3
                    ps = ps1p.tile([P, D], f32, tag="ps1", name="ps_t")
                    for j in range(4):
                        e = q0 + j
                        pass_mms(
                            ps[32 * j:32 * j + LAST, :],
                            (lambda ee: lambda kc:
                             xtiles[ee][:, kc, FULL:])(e),
                            None, tile_position=(0, 32 * j),
                        )
                    for j in range(4):
                        evac(ptk[KT - 1][:LAST, q0 + j, :],
                             ps[32 * j:32 * j + LAST, :])
                        evac(ptail[KT - 1][:LAST,
                                           (q0 + j) * LAST:
                                           (q0 + j + 1) * LAST],
                             ps[32 * j:32 * j + LAST, FULL:], small=True)

                # ---- pass 2 (main): Y_b rows, 6 full chunks ----
                # needs PT_b complete except its tail rows; tail rows of
                # ptk[KT-1] for this elem are written above only at b%4==3,
                # so elems 0..2 of each quad would read stale tail rows.
                # Instead: defer pass 2 of elems q0..q0+3 to after the quad
                # tail fill. (loop below handles it)
                if b % 4 == 3:
                    for e in range(q0, q0 + 4):
                        for ic in range(KT - 1):
                            i0 = ic * P
                            ps2 = ps2p.tile([P, D], f32, tag="ps2", name="ps2_t")
                            pass_mms(
                                ps2,
                                (lambda ee, ii: lambda kc:
                                 ptk[kc][:, ee, ii:ii + P])(e, i0),
                                P)
                            yo = yop.tile([P, D], bf16, tag="yo")
                            evac(yo[:, :], ps2[:, :])
                            nc.sync.dma_start(y[e, i0:i0 + P, :], yo[:, :])

                # prefetch after the quad block so recycling an X buffer
                # never races the quad-tail reads of older X tiles
                if b + 3 < B_LOC:
                    xtiles[b + 3] = load_x(b + 3)


            # ---- pass 2 (packed tail): all 8 elements' last 12 rows in
            # one 96-column sweep over the contiguous ptail tiles
            ps2 = ps2p.tile([P, D], f32, tag="ps2", name="ps2_t")
            pass_mms(ps2,
                     lambda kc: ptail[kc][:, :], B_LOC * LAST)
            yo = yop.tile([P, D], bf16, tag="yo")
            evac(yo[:B_LOC * LAST, :], ps2[:B_LOC * LAST, :])
            for e in range(B_LOC):
                nc.sync.dma_start(y[e, FULL:, :],
                                  yo[e * LAST:(e + 1) * LAST, :])

    nc.compile()
    return nc


def _get_program(intervals):
    key = tuple(tuple(row) for row in intervals)
    if _CACHE.get("key") != key:
        _CACHE["nc"] = _build_program(intervals)
        _CACHE["key"] = key
    return _CACHE["nc"]


def kernel(input_state, angles, A, B, C, _trace=False):
    from concourse.bass_utils import run_bass_kernel_spmd

    X = np.asarray(input_state, dtype=np.float32)
    V = _build_V(np.asarray(angles, dtype=np.float64), np.asarray(B))
    perm = _rcm_perm(V)
    inv = np.argsort(perm)
    Vp = V[np.ix_(perm, perm)]

    vt_bf = np.ascontiguousarray(Vp.T).astype(ml_dtypes.bfloat16)
    Xp = np.ascontiguousarray(X[:, perm][:, :, perm])
    X_bf = Xp.astype(ml_dtypes.bfloat16)

    intervals = _plan_intervals(Vp)
    nc = _get_program(intervals)
    in_maps = [
        {"x": X_bf[c * B_LOC:(c + 1) * B_LOC], "vt": vt_bf}
        for c in range(N_CORES)
    ]
    res = run_bass_kernel_spmd(nc, in_maps, core_ids=list(range(N_CORES)),
                               trace=_trace)
    Yp = np.concatenate([res.results[c]["y"] for c in range(N_CORES)],
                        axis=0).astype(np.float32)
    out = np.ascontiguousarray(Yp[:, inv][:, :, inv])
    if _trace:
        kernel.last_results = res
    return out


# revision 31
# speedup vs baseline: 1.0163x; 1.0163x over previous
"""Trainium2 kernel for nn_Dense_RBS_density_3D.

The reference applies 39 RBS gates sequentially to a batch of 64 density
matrices: rho <- U rho U^T. The gates compose into one orthogonal matrix
V = U_38 @ ... @ U_0, so the output is V rho V^T per batch element.

Host side: build V in fp64, then apply a reverse-Cuthill-McKee permutation of
the 780-dim basis (computed from V's significance mask) to both rows and
columns. RCM tightens V's band so each 128-row contraction tile of V^T
touches a narrower, contiguous set of output columns. Per (k-tile, PSUM bank)
the host keeps the single column interval carrying all significant mass
(mass-based threshold, DROP_BUDGET relative Frobenius perturbation).

Device side (8 NeuronCores, data-parallel over batch): per batch element
compute Y = V X V^T as two transpose-free passes of f(Z) = Z^T @ V^T
(lhsT = Z with contraction on partitions, rhs = V^T):

    Y = f(f(X))   since (X^T V^T)^T V^T = V X V^T

bf16 operands with fp32 PSUM accumulation; per-element has_written bits make
ragged-interval accumulation exact. The ragged 12-row tail chunks (780 = 6*128
+ 12) are packed: pass-1 tails of 4 elements run concurrently as 32-aligned
column-group tiles of the PE array (tile_position), and pass-2 tails of all 8
elements run as one 96-column sweep reading a strided AP over the per-k-chunk
PT tiles (which hold all 8 elements side by side).
"""

import numpy as np
import ml_dtypes

D = 780           # binom(40, 2)
N_GATES = 39
B_TOTAL = 64
N_CORES = 8
B_LOC = B_TOTAL // N_CORES
P = 128
KT = (D + P - 1) // P          # 7 k-chunks: 6x128 + 12
LAST = D - (KT - 1) * P        # 12
FULL = (KT - 1) * P            # 768
CHUNKS = [(i * P, min(P, D - i * P)) for i in range(KT)]
BANKS = [(0, 512), (512, D)]   # PSUM fp32 bank col ranges
DROP_BUDGET = 4e-3             # allowed relative Frobenius perturbation of V

_CACHE = {}


def _build_V(angles, Bmat):
    """V = U_38 @ ... @ U_0 in float64 (B[g,j,i]==+1 marks the pair)."""
    V = np.eye(D, dtype=np.float64)
    for g in range(N_GATES):
        jj, ii = np.nonzero(Bmat[g] > 0.5)
        c = np.cos(float(angles[g]))
        s = np.sin(float(angles[g]))
        Vi = V[ii, :].copy()
        Vj = V[jj, :].copy()
        V[ii, :] = c * Vi - s * Vj
        V[jj, :] = s * Vi + c * Vj
    return V


def _rcm_perm(V):
    """Bandwidth-minimizing permutation of the significance graph."""
    sq = np.sort((V ** 2).ravel())
    cs = np.cumsum(sq)
    pos = np.searchsorted(cs, (DROP_BUDGET ** 2) * cs[-1])
    thr = sq[pos - 1] if pos > 0 else -1.0
    mask = (V ** 2) > thr
    try:
        from scipy.sparse import csr_matrix
        from scipy.sparse.csgraph import reverse_cuthill_mckee
        perm = np.asarray(reverse_cuthill_mckee(csr_matrix(mask | mask.T)),
                          np.int64)
        if len(perm) == D:
            return perm
    except Exception:
        pass
    return np.arange(D)


def _plan_intervals(Vp):
    """Per (k-tile, PSUM bank): [c0, c1) column interval of Vp^T holding all
    significant mass, or None."""
    VT = Vp.T
    sliver = np.zeros((KT, D))
    for kc, (k0, ksz) in enumerate(CHUNKS):
        sliver[kc] = (VT[k0:k0 + ksz, :] ** 2).sum(axis=0)
    tot = sliver.sum()
    flat = np.sort(sliver.ravel())
    csum = np.cumsum(flat)
    pos = np.searchsorted(csum, DROP_BUDGET ** 2 * tot)
    thr = flat[pos - 1] if pos > 0 else -1.0
    sig = sliver > thr

    intervals = []
    for kc in range(KT):
        row = []
        for b0, b1 in BANKS:
            cols = np.nonzero(sig[kc, b0:b1])[0]
            if len(cols) == 0:
                row.append(None)
                continue
            c0 = int(b0 + cols[0]) & ~1
            c1 = min(b1, (int(b0 + cols[-1]) + 2) & ~1)
            row.append((c0, c1))
        intervals.append(row)

    # every column must be covered by at least one kept interval
    covered = np.zeros(D, bool)
    for row in intervals:
        for iv in row:
            if iv is not None:
                covered[iv[0]:iv[1]] = True
    if not covered.all():
        for bi, (b0, b1) in enumerate(BANKS):
            if not covered[b0:b1].all():
                kc = int(sliver[:, b0:b1].sum(axis=1).argmax())
                intervals[kc][bi] = (b0, b1)
    return intervals


def _build_program(intervals):
    import concourse.bacc as bacc
    import concourse.mybir as mybir
    import concourse.tile as tile

    nc = bacc.Bacc("TRN2", target_bir_lowering=False, debug=False,
                   num_devices=N_CORES)
    x = nc.dram_tensor("x", [B_LOC, D, D], mybir.dt.bfloat16,
                       kind="ExternalInput").ap()
    vt = nc.dram_tensor("vt", [D, D], mybir.dt.bfloat16,
                        kind="ExternalInput").ap()
    y = nc.dram_tensor("y", [B_LOC, D, D], mybir.dt.bfloat16,
                       kind="ExternalOutput").ap()

    bf16 = mybir.dt.bfloat16
    f32 = mybir.dt.float32

    kept = [(kc, bi, iv[0], iv[1])
            for kc in range(KT) for bi, iv in enumerate(intervals[kc])
            if iv is not None]
    first_kc = {}
    last_kc = {}
    for kc, bi, _, _ in kept:
        first_kc.setdefault(bi, kc)
        last_kc[bi] = kc

    with tile.TileContext(nc) as tc:
        with (
            tc.tile_pool(name="vtp", bufs=1) as vtp,
            tc.tile_pool(name="xb", bufs=6) as xbp,
            tc.tile_pool(name="pt", bufs=1) as ptp,
            tc.tile_pool(name="yo", bufs=4) as yop,
            tc.tile_pool(name="wup", bufs=1) as wup,
            tc.tile_pool(name="ps1", bufs=2, space="PSUM") as ps1p,
            tc.tile_pool(name="ps2", bufs=2, space="PSUM") as ps2p,
        ):
            # PE warmup while the first DMAs land
            wz = wup.tile([P, 512], bf16)
            nc.vector.memset(wz[:], 0.0)
            ps_w = ps1p.tile([P, D], f32, tag="ps1", name="ps_w")
            for _ in range(22):
                nc.tensor.matmul(ps_w[:, :512], wz[:, :P], wz[:, :512],
                                 start=True, stop=True)

            # V^T resident in SBUF, k-partitioned: vt_sb[p, kc, n]
            vt_sb = vtp.tile([P, KT, D], bf16)
            nc.any.memzero(vt_sb[:, KT - 1, :])
            nc.sync.dma_start(
                vt_sb[:, : KT - 1, :],
                vt[:FULL, :].rearrange("(kc p) n -> p kc n", p=P),
            )
            nc.sync.dma_start(vt_sb[:LAST, KT - 1, :], vt[FULL:, :])

            # PT mega-tiles: one per k-chunk, all 8 elements side by side.
            # ptk[kc][p, e, n] = PT_e[kc*128 + p, n]
            ptk = [ptp.tile([P, B_LOC, D], bf16, tag=f"pt{mc}",
                            name=f"ptk{mc}")
                   for mc in range(KT)]
            nc.any.memzero(ptk[KT - 1][:])   # pad partitions of ragged chunk
            # contiguous copies of PT's last-12 columns (pass-2 tail weights
            # need a single-free-dim AP): ptail[kc][p, e*12+t]
            ptail = [ptp.tile([P, B_LOC * LAST], bf16, tag=f"ptt{mc}",
                              name=f"ptail{mc}")
                     for mc in range(KT)]
            nc.any.memzero(ptail[KT - 1][:])

            copy_idx = 0

            def evac(out_ap, psum_ap, small=False):
                # alternate PSUM evacuation between ScalarE and VectorE
                nonlocal copy_idx
                if copy_idx % 2 == 0:
                    nc.scalar.copy(out_ap, psum_ap)
                else:
                    nc.vector.tensor_copy(out=out_ap, in_=psum_ap)
                copy_idx += 1

            def pass_mms(ps, src_fn, msz, tile_position=None):
                for kc, bi, c0, c1 in kept:
                    kw = {}
                    if tile_position is not None:
                        kw["tile_position"] = tile_position
                    nc.tensor.matmul(
                        ps[:, c0:c1] if msz is None else ps[:msz, c0:c1],
                        src_fn(kc),
                        vt_sb[:, kc, c0:c1],
                        start=(kc == first_kc[bi]),
                        stop=(kc == last_kc[bi]),
                        **kw,
                    )

            def load_x(b):
                xb_bf = xbp.tile([P, KT, D], bf16, tag="xb")
                nc.any.memzero(xb_bf[:, KT - 1, :])
                nc.sync.dma_start(
                    xb_bf[:, : KT - 1, :],
                    x[b, :FULL, :].rearrange("(kc p) n -> p kc n", p=P),
                )
                nc.sync.dma_start(xb_bf[:LAST, KT - 1, :], x[b, FULL:, :])
                return xb_bf

            xtiles = {}
            for b in range(min(3, B_LOC)):
                xtiles[b] = load_x(b)

            for b in range(B_LOC):
                xb_bf = xtiles[b]

                # ---- pass 1 (main): PT_b[m, :] for 6 full m-chunks ----
                for mc in range(KT - 1):
                    m0 = mc * P
                    ps = ps1p.tile([P, D], f32, tag="ps1", name="ps_t")
                    pass_mms(ps,
                             lambda kc: xb_bf[:, kc, m0:m0 + P], P)
                    evac(ptk[mc][:, b, :], ps[:, :])
                    evac(ptail[mc][:, b * LAST:(b + 1) * LAST],
                         ps[:, FULL:], small=True)

                # ---- pass 1 (tails): pack 4 elements as PE column-group
                # tiles; tails of elems 4q..4q+3 run after elem 4q+3's mains.
                if b % 4 == 3:
                    q0 = b - 3
                    ps = ps1p.tile([P, D], f32, tag="ps1", name="ps_t")
                    for j in range(4):
                        e = q0 + j
                        pass_mms(
                            ps[32 * j:32 * j + LAST, :],
                            (lambda ee: lambda kc:
                             xtiles[ee][:, kc, FULL:])(e),
                            None, tile_position=(0, 32 * j),
                        )
                    for j in range(4):
                        evac(ptk[KT - 1][:LAST, q0 + j, :],
                             ps[32 * j:32 * j + LAST, :])
                        evac(ptail[KT - 1][:LAST,
                                           (q0 + j) * LAST:
                                           (q0 + j + 1) * LAST],
                             ps[32 * j:32 * j + LAST, FULL:], small=True)

                # ---- pass 2 (main): Y_b rows, 6 full chunks ----
                # needs PT_b complete except its tail rows; tail rows of
                # ptk[KT-1] for this elem are written above only at b%4==3,
                # so elems 0..2 of each quad would read stale tail rows.
                # Instead: defer pass 2 of elems q0..q0+3 to after the quad
                # tail fill. (loop below handles it)
                if b % 4 == 3:
                    for e in range(q0, q0 + 4):
                        for ic in range(KT - 1):
                            i0 = ic * P
                            ps2 = ps2p.tile([P, D], f32, tag="ps2", name="ps2_t")
                            pass_mms(
                                ps2,
                                (lambda ee, ii: lambda kc:
                                 ptk[kc][:, ee, ii:ii + P])(e, i0),
                                P)
                            yo = yop.tile([P, D], bf16, tag="yo")
                            evac(yo[:, :], ps2[:, :])
                            nc.sync.dma_start(y[e, i0:i0 + P, :], yo[:, :])

                # prefetch after the quad block so recycling an X buffer
                # never races the quad-tail reads of older X tiles
                if b + 3 < B_LOC:
                    xtiles[b + 3] = load_x(b + 3)


            # ---- pass 2 (packed tail): all 8 elements' last 12 rows in
            # one 96-column sweep over the contiguous ptail tiles
            ps2 = ps2p.tile([P, D], f32, tag="ps2", name="ps2_t")
            pass_mms(ps2,
                     lambda kc: ptail[kc][:, :], B_LOC * LAST)
            yo = yop.tile([P, D], bf16, tag="yo")
            evac(yo[:B_LOC * LAST, :], ps2[:B_LOC * LAST, :])
            for e in range(B_LOC):
                nc.sync.dma_start(y[e, FULL:, :],
                                  yo[e * LAST:(e + 1) * LAST, :])

    nc.compile()
    return nc


def _get_program(intervals):
    key = tuple(tuple(row) for row in intervals)
    if _CACHE.get("key") != key:
        _CACHE["nc"] = _build_program(intervals)
        _CACHE["key"] = key
    return _CACHE["nc"]


def kernel(input_state, angles, A, B, C, _trace=False):
    from concourse.bass_utils import run_bass_kernel_spmd

    X = np.asarray(input_state, dtype=np.float32)
    V = _build_V(np.asarray(angles, dtype=np.float64), np.asarray(B))
    perm = _rcm_perm(V)
    inv = np.argsort(perm)
    Vp = V[np.ix_(perm, perm)]

    vt_bf = np.ascontiguousarray(Vp.T).astype(ml_dtypes.bfloat16)
    Xp = np.ascontiguousarray(X[:, perm][:, :, perm])
    X_bf = Xp.astype(ml_dtypes.bfloat16)

    intervals = _plan_intervals(Vp)
    nc = _get_program(intervals)
    in_maps = [
        {"x": X_bf[c * B_LOC:(c + 1) * B_LOC], "vt": vt_bf}
        for c in range(N_CORES)
    ]
    res = run_bass_kernel_spmd(nc, in_maps, core_ids=list(range(N_CORES)),
                               trace=_trace)
    Yp = np.concatenate([res.results[c]["y"] for c in range(N_CORES)],
                        axis=0).astype(np.float32)
    out = np.ascontiguousarray(Yp[:, inv][:, :, inv])
    if _trace:
        kernel.last_results = res
    return out
